# revision 5
# baseline (speedup 1.0000x reference)
"""Distributed Trainium2 (8 NeuronCores) kernel for masked graph attention.

Reference computation (dense masked multi-head attention over an edge set):
    q/k/v = X @ W{q,k,v} + b{q,k,v}        -> [H, N, d]
    S     = q k^T / sqrt(d)                 -> [H, N, N]
    mask  = -1e9 everywhere, 0 at edges
    P     = softmax(S + mask)               (masked entries underflow to 0.0)
    ctx   = P v                             -> [N, H*d]
    out   = ctx @ Wo + bo                   -> [N, HID]

Strategy (8 cores, row/sequence parallel):
  - Each core owns a block of N/8 query rows; K/V are computed from the
    replicated X on every core (cheaper than an all-gather).
  - Masked softmax is computed flash-style, never materializing [H,N,N] in
    HBM:  P = A * exp(S/8) where A is the 0/1 adjacency built ON DEVICE by
    GPSIMD local_scatter from per-(core, key-column) row lists.  Since
    row maxima are O(1) (scores ~ N(0, 1/3)), no max subtraction is needed;
    masked entries are exactly 0 by multiplication.
  - Scores are laid out [key j (partitions), query r (free)] so P@V contracts
    over partitions; denominators come from a 1-column ones matmul.
  - bf16 matmul inputs, fp32 PSUM accumulation, fp32 output.
"""

import contextlib
import sys
from dataclasses import dataclass

sys.path.insert(0, "/opt/trn_rl_repo")

import ml_dtypes
import numpy as np

from concourse import bacc, mybir, tile
from concourse.bass_utils import run_bass_kernel_spmd

BF16 = mybir.dt.bfloat16
F32 = mybir.dt.float32
I16 = mybir.dt.int16
AF = mybir.ActivationFunctionType
NP_BF16 = ml_dtypes.bfloat16


@dataclass(frozen=True)
class Cfg:
    N: int = 8192      # nodes
    HID: int = 256     # hidden
    H: int = 4         # heads
    CORES: int = 8
    NI: int = 32       # scatter index capacity per (core, key column)

    @property
    def D(self):       # head dim
        return self.HID // self.H

    @property
    def R(self):       # query rows per core
        return self.N // self.CORES

    @property
    def JCH(self):     # key-column chunks of 128
        return self.N // 128

    @property
    def RW(self):      # query free-dim width per score tile
        return min(512, self.R)

    @property
    def HC(self):      # hidden chunks of 128
        return self.HID // 128


CFG = Cfg()


def build(cfg: Cfg) -> bacc.Bacc:
    N, HID, H, D, R, NI = cfg.N, cfg.HID, cfg.H, cfg.D, cfg.R, cfg.NI
    JCH, RW, HC = cfg.JCH, cfg.RW, cfg.HC
    NRH = R // RW          # score tiles per (j, h)
    RQ = RW // 128         # 128-row chunks per score tile
    RC = R // 128          # 128-row chunks per core
    NCK = N // 128         # node chunks of 128
    SCALE = 1.0 / np.sqrt(D)

    nc = bacc.Bacc("TRN2", target_bir_lowering=False, debug=False,
                   num_devices=cfg.CORES)

    # ---- DRAM parameters ------------------------------------------------
    xT = nc.dram_tensor("xT", [HID, N], BF16, kind="ExternalInput")
    xTr = nc.dram_tensor("xTr", [HID, R], BF16, kind="ExternalInput")
    wq = nc.dram_tensor("wq", [HID, HID], BF16, kind="ExternalInput")
    wk = nc.dram_tensor("wk", [HID, HID], BF16, kind="ExternalInput")
    wv = nc.dram_tensor("wv", [HID, HID], BF16, kind="ExternalInput")
    wo = nc.dram_tensor("wo", [HID, HID], BF16, kind="ExternalInput")
    bq = nc.dram_tensor("bq", [HID, 1], F32, kind="ExternalInput")
    bk = nc.dram_tensor("bk", [HID, 1], F32, kind="ExternalInput")
    bv_rep = nc.dram_tensor("bv_rep", [128, HID], F32, kind="ExternalInput")
    bo_rep = nc.dram_tensor("bo_rep", [128, HID], F32, kind="ExternalInput")
    eye = nc.dram_tensor("eye", [128, 128], BF16, kind="ExternalInput")
    eidx = nc.dram_tensor("eidx", [128, JCH * NI], I16, kind="ExternalInput")
    out = nc.dram_tensor("out", [R, HID], F32, kind="ExternalOutput")

    with tile.TileContext(nc) as tc, contextlib.ExitStack() as ctx:
        cpool = ctx.enter_context(tc.tile_pool(name="const", bufs=1))
        kpool = ctx.enter_context(tc.tile_pool(name="kv", bufs=1))
        apool = ctx.enter_context(tc.tile_pool(name="mask", bufs=3))
        epool = ctx.enter_context(tc.tile_pool(name="expo", bufs=4))
        ppool = ctx.enter_context(tc.tile_pool(name="prob", bufs=4))
        opool = ctx.enter_context(tc.tile_pool(name="outs", bufs=2))
        ps_s = ctx.enter_context(tc.tile_pool(name="ps_s", bufs=2, space="PSUM"))
        ps_c = ctx.enter_context(tc.tile_pool(name="ps_c", bufs=1, space="PSUM"))
        ps_d = ctx.enter_context(tc.tile_pool(name="ps_d", bufs=1, space="PSUM"))
        ps_o = ctx.enter_context(tc.tile_pool(name="ps_o", bufs=1, space="PSUM"))

        # ---- load constants / inputs -----------------------------------
        xT_sb = [cpool.tile([128, N], BF16, name=f"xT{c}", tag=f"xT{c}") for c in range(HC)]
        xTr_sb = [cpool.tile([128, R], BF16, name=f"xTr{c}", tag=f"xTr{c}") for c in range(HC)]
        w_sb = {}
        for name, hdl in (("wq", wq), ("wk", wk), ("wv", wv), ("wo", wo)):
            w_sb[name] = [cpool.tile([128, HID], BF16, name=f"{name}{c}", tag=f"{name}{c}")
                          for c in range(HC)]
        bq_sb = [cpool.tile([128, 1], F32, name=f"bq{c}", tag=f"bq{c}") for c in range(HC)]
        bk_sb = [cpool.tile([128, 1], F32, name=f"bk{c}", tag=f"bk{c}") for c in range(HC)]
        bv_sb = cpool.tile([128, HID], F32, name="bv", tag="bv")
        bo_sb = cpool.tile([128, HID], F32, name="bo", tag="bo")
        eye_sb = cpool.tile([128, 128], BF16, name="eye", tag="eye")
        eidx_sb = cpool.tile([128, JCH * NI], I16, name="eidx", tag="eidx")
        ones_d = cpool.tile([128, NI], BF16, name="ones_d", tag="ones_d")
        ones_c = cpool.tile([128, 1], BF16, name="ones_c", tag="ones_c")

        for c in range(HC):
            nc.sync.dma_start(xT_sb[c][:], xT[c * 128:(c + 1) * 128, :])
            nc.sync.dma_start(xTr_sb[c][:], xTr[c * 128:(c + 1) * 128, :])
            for name, hdl in (("wq", wq), ("wk", wk), ("wv", wv), ("wo", wo)):
                nc.sync.dma_start(w_sb[name][c][:], hdl[c * 128:(c + 1) * 128, :])
            nc.sync.dma_start(bq_sb[c][:], bq[c * 128:(c + 1) * 128, :])
            nc.sync.dma_start(bk_sb[c][:], bk[c * 128:(c + 1) * 128, :])
        nc.sync.dma_start(bv_sb[:], bv_rep[:])
        nc.sync.dma_start(bo_sb[:], bo_rep[:])
        nc.sync.dma_start(eye_sb[:], eye[:])
        nc.sync.dma_start(eidx_sb[:], eidx[:])
        nc.vector.memset(ones_d[:], 1.0)
        nc.vector.memset(ones_c[:], 1.0)

        # ---- projections ------------------------------------------------
        # K^T [d, n] (d-chunks on partitions), Q^T [d, r], V [n, d].
        kT_sb = [kpool.tile([128, N], BF16, name=f"kT{c}", tag=f"kT{c}") for c in range(HC)]
        qT_sb = [kpool.tile([128, R], BF16, name=f"qT{c}", tag=f"qT{c}") for c in range(HC)]
        v_sb = kpool.tile([128, NCK * HID], BF16, name="v", tag="v")

        for dc in range(HC):
            for nk in range(N // 512):
                ps = ps_s.tile([128, 512], F32, name="s", tag="s")
                for hc in range(HC):
                    nc.tensor.matmul(
                        ps[:], lhsT=w_sb["wk"][hc][:, dc * 128:(dc + 1) * 128],
                        rhs=xT_sb[hc][:, nk * 512:(nk + 1) * 512],
                        start=(hc == 0), stop=(hc == HC - 1))
                nc.vector.tensor_scalar_add(
                    kT_sb[dc][:, nk * 512:(nk + 1) * 512], ps[:], bk_sb[dc][:])
            for rk in range(R // RW):
                ps = ps_s.tile([128, RW], F32, name="s", tag="s")
                for hc in range(HC):
                    nc.tensor.matmul(
                        ps[:], lhsT=w_sb["wq"][hc][:, dc * 128:(dc + 1) * 128],
                        rhs=xTr_sb[hc][:, rk * RW:(rk + 1) * RW],
                        start=(hc == 0), stop=(hc == HC - 1))
                nc.vector.tensor_scalar_add(
                    qT_sb[dc][:, rk * RW:(rk + 1) * RW], ps[:], bq_sb[dc][:])
        for nk in range(NCK):
            ps = ps_s.tile([128, HID], F32, name="s", tag="s")
            for hc in range(HC):
                nc.tensor.matmul(
                    ps[:], lhsT=xT_sb[hc][:, nk * 128:(nk + 1) * 128],
                    rhs=w_sb["wv"][hc][:],
                    start=(hc == 0), stop=(hc == HC - 1))
            nc.vector.tensor_add(v_sb[:, nk * HID:(nk + 1) * HID], ps[:], bv_sb[:])

        # ---- attention main loop over key chunks ------------------------
        # Accumulators are zero-initialized and every matmul accumulates
        # (start=False): interleaved start=True groups in one PSUM bank
        # zero a coarser granule than the addressed slice and corrupt
        # neighboring accumulation groups.
        c_ps = [ps_c.tile([128, RC * D], F32, name=f"c{h}", tag=f"c{h}") for h in range(H)]
        d_ps = ps_d.tile([128, H * RC], F32, name="d", tag="d")
        for h in range(H):
            nc.vector.memset(c_ps[h][:], 0.0)
        nc.vector.memset(d_ps[:], 0.0)

        for j in range(JCH):
            a_t = apool.tile([128, R], BF16, name="a", tag="a")
            nc.gpsimd.local_scatter(
                a_t[:], ones_d[:], eidx_sb[:, j * NI:(j + 1) * NI],
                channels=128, num_elems=R, num_idxs=NI)
            for h in range(H):
                hc, hp = h // 2, (h % 2) * D
                for rh in range(NRH):
                    s_ps = ps_s.tile([128, RW], F32, name="s", tag="s")
                    nc.tensor.matmul(
                        s_ps[:],
                        lhsT=kT_sb[hc][hp:hp + D, j * 128:(j + 1) * 128],
                        rhs=qT_sb[hc][hp:hp + D, rh * RW:(rh + 1) * RW],
                        start=True, stop=True)
                    e_t = epool.tile([128, RW], BF16, name="e", tag="e")
                    nc.scalar.activation(e_t[:], s_ps[:], AF.Exp, scale=SCALE)
                    p_t = ppool.tile([128, RW], BF16, name="p", tag="p")
                    nc.vector.tensor_mul(
                        p_t[:], e_t[:], a_t[:, rh * RW:(rh + 1) * RW])
                    for rq in range(RQ):
                        rc = rh * RQ + rq
                        nc.tensor.matmul(
                            c_ps[h][:, rc * D:(rc + 1) * D],
                            lhsT=p_t[:, rq * 128:(rq + 1) * 128],
                            rhs=v_sb[:, j * HID + h * D: j * HID + h * D + D],
                            start=False, stop=(j == JCH - 1),
                            skip_group_check=True)
                        nc.tensor.matmul(
                            d_ps[:, h * RC + rc: h * RC + rc + 1],
                            lhsT=p_t[:, rq * 128:(rq + 1) * 128],
                            rhs=ones_c[:],
                            start=False, stop=(j == JCH - 1),
                            skip_group_check=True)

        # ---- epilogue: normalize, transpose, output projection ----------
        recip = cpool.tile([128, H * RC], F32, name="recip", tag="recip")
        nc.vector.reciprocal(recip[:], d_ps[:])

        ctxT_sb = [kpool.tile([128, R], BF16, name=f"ctxT{c}", tag=f"ctxT{c}") for c in range(HC)]
        for rc in range(RC):
            ctx_n = opool.tile([128, HID], BF16, name="ctxn", tag="ctxn")
            for h in range(H):
                nc.vector.tensor_scalar_mul(
                    ctx_n[:, h * D:(h + 1) * D],
                    c_ps[h][:, rc * D:(rc + 1) * D],
                    recip[:, h * RC + rc: h * RC + rc + 1])
            for hc in range(HC):
                tr = ps_s.tile([128, 128], BF16, name="tr", tag="s")
                nc.tensor.transpose(
                    tr[:], ctx_n[:, hc * 128:(hc + 1) * 128], eye_sb[:])
                nc.vector.tensor_copy(
                    ctxT_sb[hc][:, rc * 128:(rc + 1) * 128], tr[:])
        for rc in range(RC):
            po = ps_o.tile([128, HID], F32, name="o", tag="o")
            for hc in range(HC):
                nc.tensor.matmul(
                    po[:], lhsT=ctxT_sb[hc][:, rc * 128:(rc + 1) * 128],
                    rhs=w_sb["wo"][hc][:],
                    start=(hc == 0), stop=(hc == HC - 1))
            osb = opool.tile([128, HID], F32, name="osb", tag="osb")
            nc.vector.tensor_add(osb[:], po[:], bo_sb[:])
            nc.sync.dma_start(out[rc * 128:(rc + 1) * 128, :], osb[:])

    nc.compile()
    return nc


# -------------------------------------------------------------------------
# Host-side input prep / sharding
# -------------------------------------------------------------------------

def prep_in_maps(cfg: Cfg, node_features, Wq, bq, Wk, bk, Wv, bv, Wo, bo,
                 edge_index):
    N, HID, R, NI, JCH = cfg.N, cfg.HID, cfg.R, cfg.NI, cfg.JCH
    x = np.asarray(node_features, np.float32)
    xT16 = np.ascontiguousarray(x.T).astype(NP_BF16)

    r = np.asarray(edge_index[0], np.int64)
    c = np.asarray(edge_index[1], np.int64)
    lin = np.unique(r * N + c)                 # dedup (reference scatter-set)
    ur, uc = lin // N, lin % N

    key = (ur // R) * N + uc                   # group by (core, key column)
    order = np.argsort(key, kind="stable")
    ks = key[order]
    rows_local = (ur % R)[order].astype(np.int16)
    grp_start = np.r_[0, np.flatnonzero(np.diff(ks)) + 1]
    grp_len = np.diff(np.r_[grp_start, len(ks)])
    idx_in_grp = np.arange(len(ks)) - np.repeat(grp_start, grp_len)
    assert idx_in_grp.max() < NI, f"edge fan-in {idx_in_grp.max()+1} > NI={NI}"
    core_g = ks // N
    col_g = ks % N
    eidx = np.full((cfg.CORES, 128, JCH * NI), -1, np.int16)
    eidx[core_g, col_g % 128, (col_g // 128) * NI + idx_in_grp] = rows_local

    common = {
        "xT": xT16,
        "wq": np.asarray(Wq, np.float32).astype(NP_BF16),
        "wk": np.asarray(Wk, np.float32).astype(NP_BF16),
        "wv": np.asarray(Wv, np.float32).astype(NP_BF16),
        "wo": np.asarray(Wo, np.float32).astype(NP_BF16),
        "bq": np.asarray(bq, np.float32).reshape(HID, 1),
        "bk": np.asarray(bk, np.float32).reshape(HID, 1),
        "bv_rep": np.broadcast_to(np.asarray(bv, np.float32), (128, HID)).copy(),
        "bo_rep": np.broadcast_to(np.asarray(bo, np.float32), (128, HID)).copy(),
        "eye": np.eye(128, dtype=NP_BF16),
    }
    in_maps = []
    for core in range(cfg.CORES):
        m = dict(common)
        m["xTr"] = np.ascontiguousarray(xT16[:, core * R:(core + 1) * R])
        m["eidx"] = eidx[core]
        in_maps.append(m)
    return in_maps


_CACHE = {}


def _get_nc(cfg: Cfg):
    if cfg not in _CACHE:
        _CACHE[cfg] = build(cfg)
    return _CACHE[cfg]


def run(cfg: Cfg, **inputs) -> np.ndarray:
    nc = _get_nc(cfg)
    in_maps = prep_in_maps(cfg, **inputs)
    res = run_bass_kernel_spmd(nc, in_maps, core_ids=list(range(cfg.CORES)))
    return np.concatenate(
        [np.asarray(res.results[i]["out"], np.float32)
         for i in range(cfg.CORES)], axis=0)


def kernel(**inputs) -> np.ndarray:
    return run(CFG, **inputs)


# -------------------------------------------------------------------------
# Self-test at reduced scale (numpy oracle)
# -------------------------------------------------------------------------

def _ref_np(cfg: Cfg, node_features, Wq, bq, Wk, bk, Wv, bv, Wo, bo,
            edge_index):
    N, H, D = cfg.N, cfg.H, cfg.D
    x = np.asarray(node_features, np.float64)
    q = (x @ Wq + bq).reshape(N, H, D).transpose(1, 0, 2)
    k = (x @ Wk + bk).reshape(N, H, D).transpose(1, 0, 2)
    v = (x @ Wv + bv).reshape(N, H, D).transpose(1, 0, 2)
    s = np.einsum("hnd,hmd->hnm", q, k) / np.sqrt(D)
    mask = np.full((N, N), -1e9)
    mask[edge_index[0], edge_index[1]] = 0.0
    s = s + mask[None]
    s = s - s.max(-1, keepdims=True)
    p = np.exp(s)
    p /= p.sum(-1, keepdims=True)
    ctx2 = np.einsum("hnm,hmd->hnd", p, v).transpose(1, 0, 2).reshape(N, H * D)
    return ctx2 @ Wo + bo


def _selftest(cfg: Cfg):
    rng = np.random.default_rng(0)
    N, HID = cfg.N, cfg.HID
    s = 1.0 / np.sqrt(HID)
    inp = dict(
        node_features=rng.standard_normal((N, HID)).astype(np.float32),
        Wq=rng.uniform(-s, s, (HID, HID)).astype(np.float32),
        bq=rng.uniform(-0.1, 0.1, (HID,)).astype(np.float32),
        Wk=rng.uniform(-s, s, (HID, HID)).astype(np.float32),
        bk=rng.uniform(-0.1, 0.1, (HID,)).astype(np.float32),
        Wv=rng.uniform(-s, s, (HID, HID)).astype(np.float32),
        bv=rng.uniform(-0.1, 0.1, (HID,)).astype(np.float32),
        Wo=rng.uniform(-s, s, (HID, HID)).astype(np.float32),
        bo=rng.uniform(-0.1, 0.1, (HID,)).astype(np.float32),
        edge_index=rng.integers(0, N, (2, N * 32)).astype(np.int64),
    )
    got = run(cfg, **inp)
    want = _ref_np(cfg, **inp)
    err = np.abs(got - want.astype(np.float32))
    denom = np.abs(want).max()
    rel = np.linalg.norm(got - want) / np.linalg.norm(want)
    print(f"selftest N={cfg.N}: max_abs={err.max():.4e} "
          f"absmax_scale={denom:.3e} rel_fro={rel:.4e}")
    return rel


if __name__ == "__main__":
    mini = Cfg(N=1024, HID=256, H=4, CORES=8, NI=32)
    _selftest(mini)
